# revision 1
# baseline (speedup 1.0000x reference)
"""GQA attention (S=2048, D=4096, 32 Q heads / 8 KV heads, RoPE, full attn)
distributed over 8 Trainium2 NeuronCores.

Strategy (tensor-parallel by heads, Megatron-style with an AllGather before
the output projection instead of an AllReduce after it):
  - core c owns Q heads 4c..4c+3 and KV head c (GQA groups align with cores).
  - projections computed as transposed GEMMs: QT/KT [chan, tok] directly
    usable by the scores matmul; V via VT + PE transposes.
  - RoPE folded into two PE "mix" matmuls over host-deinterleaved channels
    (evens then odds per head), scale folded into wq on the host.
  - scores computed transposed, ST = KT.T @ QT -> [k, q]: softmax normalizer
    via DVE partial sums + one ones-matmul; exp on ScalarE (f32, near-exact);
    PV matmul takes V as stationary operand and E as moving operand.
  - attnT [512, 2048] normalized, AllGathered per 512-token q-chunk (4
    pipelined AllGathers), then each core computes a 512-row slice of the
    transposed output projection finalT = woT.T @ attnT_full.
  - all matmuls run in float32r (bf16-speed, ~1.5e-4 matmul error).

Host side only reshapes/transposes/pads and concatenates outputs.
"""
import sys

import numpy as np
import ml_dtypes

_BF16 = ml_dtypes.bfloat16

for _p in ("/root/.axon_site/_ro/trn_rl_repo", "/opt/trn_rl_repo"):
    if _p not in sys.path:
        sys.path.append(_p)

import concourse.bass as bass
import concourse.tile as tile
from concourse import mybir
from concourse.bass_utils import run_bass_kernel_spmd

N_CORES = 8
S = 2048
D = 4096
HD = 128
N_QH = 4          # Q heads per core
N_KT = S // 128   # 16 k-tiles
N_TC = S // 512   # 4 token chunks
N_KC = D // 128   # 32 contraction tiles
F32 = mybir.dt.float32
F32R = mybir.dt.float32r

_NC_CACHE = {}


def _bc(ap):
    return ap.bitcast(F32R)


def _split_multi_waits(nc):
    """This container's walrus accepts only ONE sync-wait per instruction
    encoding; hoist extra waits onto fresh single-wait NoOps placed before
    the instruction on the same engine."""
    n = 0
    for fn in nc.m.functions:
        for bb in fn.blocks:
            new_insts = []
            changed = False
            for ins in bb.instructions:
                si = ins.sync_info
                waits = list(si.on_wait) if si is not None else []
                if len(waits) > 1:
                    for w in waits[:-1]:
                        n += 1
                        nop = mybir.InstNoOp(name=f"WSPL-{n}", ins=[], outs=[])
                        nop.engine = ins.engine
                        nop.sync_info = mybir.SyncInfo(on_wait=[w], on_update=[])
                        new_insts.append(nop)
                    si.on_wait = waits[-1:]
                    changed = True
                new_insts.append(ins)
            if changed:
                bb.instructions = new_insts
    return n


def _build():
    nc = bass.Bass()

    xt = nc.dram_tensor("xt", [D, S], F32R, kind="ExternalInput")
    wqt = nc.dram_tensor("wqt", [D, 512], F32R, kind="ExternalInput")
    wkt = nc.dram_tensor("wkt", [D, HD], F32R, kind="ExternalInput")
    wvt = nc.dram_tensor("wvt", [D, HD], F32R, kind="ExternalInput")
    wot = nc.dram_tensor("wot", [D, 512], mybir.dt.bfloat16, kind="ExternalInput")
    cs1 = nc.dram_tensor("cs1", [HD, S], F32, kind="ExternalInput")
    cs2 = nc.dram_tensor("cs2", [HD, S], F32, kind="ExternalInput")
    mix1 = nc.dram_tensor("mix1", [HD, HD], F32R, kind="ExternalInput")
    mix2 = nc.dram_tensor("mix2", [HD, HD], F32R, kind="ExternalInput")
    onesc = nc.dram_tensor("onesc", [HD, 1], F32R, kind="ExternalInput")
    onesr = nc.dram_tensor("onesr", [1, HD], F32R, kind="ExternalInput")
    ident = nc.dram_tensor("ident", [HD, HD], F32R, kind="ExternalInput")
    out_ext = nc.dram_tensor("out", [S, 512], F32, kind="ExternalOutput")

    ag_in = [
        nc.dram_tensor(f"agi{qc}", [512, 512], mybir.dt.bfloat16) for qc in range(N_TC)
    ]
    ag_out = [
        nc.dram_tensor(f"ago{qc}", [D, 512], mybir.dt.bfloat16, addr_space="Shared")
        for qc in range(N_TC)
    ]

    xt_r = xt.rearrange("(kc p) s -> kc p s", p=128)
    wqt_r = wqt.rearrange("(kc p) n -> kc p n", p=128)
    wkt_r = wkt.rearrange("(kc p) n -> kc p n", p=128)
    wvt_r = wvt.rearrange("(kc p) n -> kc p n", p=128)
    wot_r = wot.rearrange("(hk p) n -> hk p n", p=128)

    with tile.TileContext(nc) as tc:
        with (
            tc.tile_pool(name="const", bufs=1) as constp,
            tc.tile_pool(name="persist", bufs=1) as persist,
        ):
            # constants
            cs1_sb = constp.tile([HD, S], F32)
            cs2_sb = constp.tile([HD, S], F32)
            mix1_sb = constp.tile([HD, HD], F32R)
            mix2_sb = constp.tile([HD, HD], F32R)
            onesc_sb = constp.tile([HD, 1], F32R)
            onesr_sb = constp.tile([1, HD], F32R)
            ident_sb = constp.tile([HD, HD], F32R)
            nc.gpsimd.dma_start(out=cs1_sb[:], in_=cs1[:])
            nc.gpsimd.dma_start(out=cs2_sb[:], in_=cs2[:])
            nc.gpsimd.dma_start(out=mix1_sb[:], in_=mix1[:])
            nc.gpsimd.dma_start(out=mix2_sb[:], in_=mix2[:])
            nc.gpsimd.dma_start(out=onesc_sb[:], in_=onesc[:])
            nc.gpsimd.dma_start(out=onesr_sb[:], in_=onesr[:])
            nc.gpsimd.dma_start(out=ident_sb[:], in_=ident[:])

            # persistent activations
            qt_sb = persist.tile([128, N_QH, S], F32R)   # QT_rope
            kt_sb = persist.tile([128, S], F32R)         # KT_rope
            v_sb = persist.tile([128, N_KT, HD], F32R)   # V [tok-in-tile, kt, chan]

            # ---------------- phase 1: projections + rope ----------------
            with (
                tc.tile_pool(name="wq", bufs=1) as wqp,
                tc.tile_pool(name="xtp", bufs=3) as xtp,
                tc.tile_pool(name="uv", bufs=2) as uvp,
                tc.tile_pool(name="vt", bufs=2) as vtp,
                tc.tile_pool(name="p1q", bufs=1, space="PSUM") as p1q,
                tc.tile_pool(name="p1k", bufs=1, space="PSUM") as p1k,
                tc.tile_pool(name="p1r", bufs=1, space="PSUM") as p1r,
            ):
                wq_sb = wqp.tile([128, N_KC, 512], F32R)

                def _load_wq_chunk(ch):
                    nc.sync.dma_start(
                        out=wq_sb[:, ch * 8:(ch + 1) * 8, :],
                        in_=wqt_r[ch * 8:(ch + 1) * 8].rearrange("kc p n -> p kc n"),
                    )

                _load_wq_chunk(0)
                wk_sb = wqp.tile([128, N_KC, HD], F32R)
                nc.sync.dma_start(out=wk_sb[:], in_=wkt_r[:].rearrange("kc p n -> p kc n"))
                wv_sb = wqp.tile([128, N_KC, HD], F32R)
                nc.sync.dma_start(out=wv_sb[:], in_=wvt_r[:].rearrange("kc p n -> p kc n"))

                for tcb in range(N_TC):
                    t0 = tcb * 512
                    scope = nc.named_scope(f"proj{tcb}"); scope.__enter__()
                    qps = [
                        p1q.tile([128, 512], F32, name=f"qps{tcb}_{h}", tag=f"qps{h}")
                        for h in range(N_QH)
                    ]
                    kps = p1k.tile([128, 512], F32, name=f"kps{tcb}", tag="kps")
                    vtps = p1k.tile([128, 512], F32, name=f"vtps{tcb}", tag="vtps")
                    xt_g = None
                    for kc in range(N_KC):
                        if kc % 4 == 0:
                            xt_g = xtp.tile([128, 4, 512], F32R, name=f"xt{tcb}_{kc}", tag="xt")
                            nc.sync.dma_start(
                                out=xt_g[:],
                                in_=xt_r[kc:kc + 4, :, t0:t0 + 512].rearrange("g p n -> p g n"),
                            )
                            if tcb == 0 and kc in (4, 8, 12):
                                _load_wq_chunk(kc // 4)
                        xt_t = xt_g[:, kc % 4, :]
                        st, sp = kc == 0, kc == N_KC - 1
                        for h in range(N_QH):
                            nc.tensor.matmul(
                                qps[h][:], wq_sb[:, kc, h * 128:(h + 1) * 128],
                                xt_t, start=st, stop=sp,
                            )
                        nc.tensor.matmul(kps[:], wk_sb[:, kc, :], xt_t, start=st, stop=sp)
                        nc.tensor.matmul(vtps[:], wv_sb[:, kc, :], xt_t, start=st, stop=sp)

                    # rope for Q heads and K of this token chunk
                    for h in range(N_QH + 1):
                        src = kps if h == N_QH else qps[h]
                        u_t = uvp.tile([128, 512], F32R, name=f"u{tcb}_{h}", tag="u")
                        v_t = uvp.tile([128, 512], F32R, name=f"v{tcb}_{h}", tag="v")
                        nc.vector.tensor_mul(u_t[:], src[:], cs1_sb[:, t0:t0 + 512])
                        nc.vector.tensor_mul(v_t[:], src[:], cs2_sb[:, t0:t0 + 512])
                        rps = p1r.tile([128, 512], F32, name=f"rps{tcb}_{h}", tag="rps")
                        nc.tensor.matmul(rps[:], mix1_sb[:], u_t[:], start=True, stop=False)
                        nc.tensor.matmul(rps[:], mix2_sb[:], v_t[:], start=False, stop=True)
                        if h == N_QH:
                            nc.vector.tensor_copy(kt_sb[:, t0:t0 + 512], rps[:])
                        else:
                            nc.vector.tensor_copy(qt_sb[:, h, t0:t0 + 512], rps[:])

                    # V for this token chunk: VT -> PE transpose -> V
                    vt_sb = vtp.tile([128, 512], F32R, name=f"vts{tcb}", tag="vts")
                    nc.vector.tensor_copy(vt_sb[:], vtps[:])
                    vtr = p1r.tile([128, 4, 128], F32R, name=f"vtr{tcb}", tag="vtr")
                    for j in range(4):
                        nc.tensor.transpose(
                            vtr[:, j, :], vt_sb[:, j * 128:(j + 1) * 128],
                            ident_sb[:],
                        )
                    nc.vector.tensor_copy(v_sb[:, tcb * 4:(tcb + 1) * 4, :], vtr[:])
                    scope.__exit__(None, None, None)

            # ------------- phase 2+3: attention + AllGather per q-chunk -----
            # ------------- phase 4: wo GEMM per q-chunk ---------------------
            with (
                tc.tile_pool(name="wo", bufs=1) as wop,
                tc.tile_pool(name="ep", bufs=4) as ep,
                tc.tile_pool(name="zp", bufs=1) as zp,
                tc.tile_pool(name="np_", bufs=2) as np_,
                tc.tile_pool(name="agp", bufs=5) as agp,
                tc.tile_pool(name="fout", bufs=2) as foutp,
                tc.tile_pool(name="p2s", bufs=2, space="PSUM") as p2s,
                tc.tile_pool(name="p2pv", bufs=1, space="PSUM") as p2pv,
                tc.tile_pool(name="p4f", bufs=1, space="PSUM") as p4f,
            ):
                wo_sb = wop.tile([128, N_KC, 512], mybir.dt.bfloat16)
                nc.sync.dma_start(out=wo_sb[:], in_=wot_r[:].rearrange("hk p n -> p hk n"))

                for qc in range(N_TC):
                    q0 = qc * 512
                    scope = nc.named_scope(f"attn{qc}"); scope.__enter__()
                    for hp in range(2):
                      hs = [2 * hp, 2 * hp + 1]
                      pvs = {}
                      for h in hs:
                          pvs[h] = p2pv.tile([128, 512], F32, name=f"pv{qc}_{h}", tag=f"pv{h % 2}")
                      zparts = {}
                      for h in hs:
                          zparts[h] = zp.tile([128, 512], F32R, name=f"zpt{qc}_{h}", tag=f"zpart{h % 2}")
                      for kt in range(N_KT):
                        k0 = kt * 128
                        stp = p2s.tile([128, 2, 512], F32, name=f"st{qc}_{hp}_{kt}", tag="st")
                        for j, h in enumerate(hs):
                            nc.tensor.matmul(
                                stp[:, j, :], kt_sb[:, k0:k0 + 128],
                                qt_sb[:, h, q0:q0 + 512], start=True, stop=True,
                            )
                        e_t = ep.tile([128, 2, 512], F32R, name=f"e{qc}_{kt}_{hp}", tag="e")
                        nc.scalar.activation(
                            out=e_t[:], in_=stp[:],
                            func=mybir.ActivationFunctionType.Exp,
                        )
                        for j, h in enumerate(hs):
                            if kt == 0:
                                nc.vector.tensor_copy(zparts[h][:], e_t[:, j, :])
                            else:
                                nc.vector.tensor_add(zparts[h][:], zparts[h][:], e_t[:, j, :])
                        for j, h in enumerate(hs):
                            nc.tensor.matmul(
                                pvs[h][:], v_sb[:, kt, :], e_t[:, j, :],
                                start=(kt == 0), stop=(kt == N_KT - 1),
                            )
                      for h in hs:
                        pvc = np_.tile([128, 512], F32, name=f"pvc{qc}_{h}", tag="pvc")
                        nc.vector.tensor_copy(pvc[:], pvs[h][:])
                        zps = p2s.tile([1, 512], F32, name=f"zps{qc}_{h}", tag="st")
                        nc.tensor.matmul(zps[:], onesc_sb[:], zparts[h][:], start=True, stop=True)
                        lnz = zp.tile([1, 512], F32, name=f"lnz{qc}_{h}", tag="lnz")
                        nc.scalar.activation(
                            out=lnz[:], in_=zps[:],
                            func=mybir.ActivationFunctionType.Ln,
                        )
                        invzr = zp.tile([1, 512], F32R, name=f"izr{qc}_{h}", tag="invzr")
                        nc.scalar.activation(
                            out=invzr[:], in_=lnz[:],
                            func=mybir.ActivationFunctionType.Exp, scale=-1.0,
                        )
                        bcps = p2s.tile([128, 512], F32, name=f"bc{qc}_{h}", tag="st")
                        nc.tensor.matmul(bcps[:], onesr_sb[:], invzr[:], start=True, stop=True)
                        bc_sb = np_.tile([128, 512], F32, name=f"bcs{qc}_{h}", tag="bcs")
                        nc.vector.tensor_copy(bc_sb[:], bcps[:])
                        at_sb = np_.tile([128, 512], mybir.dt.bfloat16, name=f"at{qc}_{h}", tag="at")
                        nc.vector.tensor_mul(at_sb[:], pvc[:], bc_sb[:])
                        nc.sync.dma_start(
                            out=ag_in[qc][h * 128:(h + 1) * 128, :], in_=at_sb[:]
                        )

                    scope.__exit__(None, None, None)
                    scope = nc.named_scope(f"ag{qc}"); scope.__enter__()
                    nc.gpsimd.collective_compute(
                        "AllGather",
                        mybir.AluOpType.bypass,
                        replica_groups=[list(range(N_CORES))],
                        ins=[ag_in[qc][:].opt()],
                        outs=[ag_out[qc][:].opt()],
                    )

                    scope.__exit__(None, None, None)
                    scope = nc.named_scope(f"wo{qc}"); scope.__enter__()
                    # wo GEMM for this q-chunk (natural orientation: out[q, d]).
                    # rhs tiles DMA'd in 8-tile groups (fewer, bigger DMAs);
                    # all resident; two passes of q-subtile pairs (2 PSUM banks).
                    ago_r = ag_out[qc].rearrange("(hk p) n -> hk p n", p=128)
                    rhs_gs = []
                    for g in range(4):
                        rhs_g = agp.tile([128, 8, 512], mybir.dt.bfloat16, name=f"ag{qc}_{g}", tag="ag")
                        nc.gpsimd.dma_start(
                            out=rhs_g[:],
                            in_=ago_r[g * 8:(g + 1) * 8].rearrange("g p n -> p g n"),
                        )
                        rhs_gs.append(rhs_g)
                    for qsp in range(2):
                        fps = [
                            p4f.tile([128, 512], F32, name=f"f{qc}_{qsp}_{j}", tag=f"f{j}")
                            for j in range(2)
                        ]
                        for hk in range(N_KC):
                            for j in range(2):
                                qs = qsp * 2 + j
                                nc.tensor.matmul(
                                    fps[j][:],
                                    rhs_gs[hk // 8][:, hk % 8, qs * 128:(qs + 1) * 128],
                                    wo_sb[:, hk, :], start=(hk == 0), stop=(hk == N_KC - 1),
                                )
                        for j in range(2):
                            qs = qsp * 2 + j
                            f_sb = foutp.tile([128, 512], F32, name=f"fs{qc}_{qs}", tag="fs")
                            nc.vector.tensor_copy(f_sb[:], fps[j][:])
                            nc.gpsimd.dma_start(
                                out=out_ext[q0 + qs * 128:q0 + (qs + 1) * 128, :],
                                in_=f_sb[:],
                            )
                    scope.__exit__(None, None, None)

    _split_multi_waits(nc)
    return nc


def _host_prep(x, cos, sin, wq, wk, wv, wo):
    scale = np.float32(HD ** -0.5)
    perm = np.concatenate([np.arange(0, HD, 2), np.arange(1, HD, 2)])

    xT = np.ascontiguousarray(x.T)
    cosT = np.ascontiguousarray(cos.T)
    sinT = np.ascontiguousarray(sin.T)
    cs1 = np.concatenate([cosT, sinT], axis=0)
    cs2 = np.concatenate([sinT, cosT], axis=0)

    m1 = np.zeros((HD, HD), np.float32)
    m1[np.arange(64), np.arange(64)] = 1.0
    m1[np.arange(64) + 64, np.arange(64)] = -1.0
    m2 = np.zeros((HD, HD), np.float32)
    m2[np.arange(64), np.arange(64) + 64] = 1.0
    m2[np.arange(64) + 64, np.arange(64) + 64] = 1.0

    shared = {
        "xt": xT,
        "cs1": cs1,
        "cs2": cs2,
        "mix1": m1,
        "mix2": m2,
        "onesc": np.ones((HD, 1), np.float32),
        "onesr": np.ones((1, HD), np.float32),
        "ident": np.eye(HD, dtype=np.float32),
    }
    in_maps = []
    for c in range(N_CORES):
        wq_c = wq[c * 512:(c + 1) * 512].reshape(N_QH, HD, D)[:, perm, :]
        wq_c = (wq_c.reshape(512, D) * scale)
        wk_c = wk[c * HD:(c + 1) * HD][perm, :]
        wv_c = wv[c * HD:(c + 1) * HD]
        wo_c = wo[c * 512:(c + 1) * 512]
        m = dict(shared)
        m["wqt"] = np.ascontiguousarray(wq_c.T)
        m["wkt"] = np.ascontiguousarray(wk_c.T)
        m["wvt"] = np.ascontiguousarray(wv_c.T)
        m["wot"] = np.ascontiguousarray(wo_c.T).astype(_BF16)
        in_maps.append(m)
    return in_maps


def kernel(x, cos, sin, wq, wk, wv, wo, _trace=False):
    x = np.asarray(x, np.float32)
    cos = np.asarray(cos, np.float32)
    sin = np.asarray(sin, np.float32)
    wq = np.asarray(wq, np.float32)
    wk = np.asarray(wk, np.float32)
    wv = np.asarray(wv, np.float32)
    wo = np.asarray(wo, np.float32)

    in_maps = _host_prep(x, cos, sin, wq, wk, wv, wo)
    if "nc" not in _NC_CACHE:
        _NC_CACHE["nc"] = _build()
    nc = _NC_CACHE["nc"]
    res = run_bass_kernel_spmd(
        nc, in_maps, core_ids=list(range(N_CORES)), trace=_trace
    )
    out = np.concatenate([res.results[c]["out"] for c in range(N_CORES)], axis=1)
    out = np.ascontiguousarray(out, dtype=np.float32)
    if _trace:
        kernel._last_exec_time_ns = res.exec_time_ns
        kernel._last_result = res
    return out



# revision 16
# speedup vs baseline: 1.1246x; 1.1246x over previous
"""GQA attention (S=2048, D=4096, 32 Q heads / 8 KV heads, RoPE, full attn)
distributed over 8 Trainium2 NeuronCores.

Strategy (tensor-parallel by heads; AllGather of normalized attention before
the output projection, split per head-pair for pipelining):
  - core c owns Q heads 4c..4c+3 and KV head c (GQA groups align with cores).
  - all GEMMs in bf16 (stationary and moving), f32 PSUM accumulation.
  - projections as transposed GEMMs QT/KT/VT [chan, tok] via staggered
    per-output sweeps (one PSUM bank per output, 32 contraction MMs each),
    so banks release one at a time and RoPE drains overlap the next sweep.
  - RoPE on DVE only: u = p*cs1, v = p*cs2 (PSUM reads), then
    half-partition sub/add writes the bf16 QT/KT tiles. attn scale folded
    into wq host-side; channels deinterleaved (evens;odds) host-side.
  - V via a transposing SBUF->SBUF DMA instead of PE transposes.
  - attention per 512-token q-chunk, per head: scores ST=[k,q] per k-tile,
    exp on ScalarE ([128,512] -> bf16), z accumulated on DVE (running f32
    [128,512] + one ones-matmul partition reduce), PV accumulated in PSUM
    across all 16 k-tiles, normalize via DVE reciprocal + broadcast matmul.
  - the attention inner loop is ScalarE(exp)-paced, so independent "filler"
    matmuls (wo-GEMM quarters of earlier chunks, or the last chunk's Q
    projection sweeps) are interleaved between score/PV pairs to keep the
    in-order PE queue busy during exp waits.
  - AllGather per (chunk, head-pair): 8 collectives of [256,512]bf16 in,
    [2048,512]bf16 out; wo GEMM accumulates over the two gathered waves,
    2 PSUM banks (q-subtile pairs), quarters scheduled one head-slot late
    so each AG has ~2 head-slots of latency slack before first use.
Host side only reshapes/transposes/casts inputs and concatenates outputs.
"""
import sys

import numpy as np
import ml_dtypes

_BF16 = ml_dtypes.bfloat16

for _p in ("/root/.axon_site/_ro/trn_rl_repo", "/opt/trn_rl_repo"):
    if _p not in sys.path:
        sys.path.append(_p)

import concourse.bass as bass
import concourse.tile as tile
from concourse import mybir
from concourse.bass_utils import run_bass_kernel_spmd

N_CORES = 8
S = 2048
D = 4096
HD = 128
N_QH = 4          # Q heads per core
N_KT = S // 128   # 16 k-tiles
N_TC = S // 512   # 4 token chunks
N_KC = D // 128   # 32 contraction tiles
F32 = mybir.dt.float32
BF = mybir.dt.bfloat16

_NC_CACHE = {}


def _split_multi_waits(nc):
    """This container's walrus accepts only ONE sync-wait per instruction
    encoding; hoist extra waits onto fresh single-wait NoOps placed before
    the instruction on the same engine."""
    n = 0
    for fn in nc.m.functions:
        for bb in fn.blocks:
            new_insts = []
            changed = False
            for ins in bb.instructions:
                si = ins.sync_info
                waits = list(si.on_wait) if si is not None else []
                if len(waits) > 1:
                    for w in waits[:-1]:
                        n += 1
                        nop = mybir.InstNoOp(name=f"WSPL-{n}", ins=[], outs=[])
                        nop.engine = ins.engine
                        nop.sync_info = mybir.SyncInfo(on_wait=[w], on_update=[])
                        new_insts.append(nop)
                    si.on_wait = waits[-1:]
                    changed = True
                new_insts.append(ins)
            if changed:
                bb.instructions = new_insts
    return n


def _build():
    nc = bass.Bass()

    xt = nc.dram_tensor("xt", [N_KC, 128, S], BF, kind="ExternalInput")
    wqt = nc.dram_tensor("wqt", [128, N_KC, 512], BF, kind="ExternalInput")
    wkt = nc.dram_tensor("wkt", [128, N_KC, HD], BF, kind="ExternalInput")
    wvt = nc.dram_tensor("wvt", [128, N_KC, HD], BF, kind="ExternalInput")
    wot = nc.dram_tensor("wot", [128, N_KC, 512], BF, kind="ExternalInput")
    cs1 = nc.dram_tensor("cs1", [HD, S], BF, kind="ExternalInput")
    cs2 = nc.dram_tensor("cs2", [HD, S], BF, kind="ExternalInput")
    mix1 = nc.dram_tensor("mix1", [HD, HD], BF, kind="ExternalInput")
    mix2 = nc.dram_tensor("mix2", [HD, HD], BF, kind="ExternalInput")
    onesc = nc.dram_tensor("onesc", [HD, 1], F32, kind="ExternalInput")
    onesr = nc.dram_tensor("onesr", [1, HD], F32, kind="ExternalInput")
    out_ext = nc.dram_tensor("out", [S, 512], F32, kind="ExternalOutput")

    ag_in = [
        [nc.dram_tensor(f"agi{qc}_{hp}", [256, 512], BF) for hp in range(2)]
        for qc in range(N_TC)
    ]
    ag_out = [
        [
            nc.dram_tensor(f"ago{qc}_{hp}", [2048, 512], BF, addr_space="Shared")
            for hp in range(2)
        ]
        for qc in range(N_TC)
    ]

    with tile.TileContext(nc) as tc:
        with (
            tc.tile_pool(name="const", bufs=1) as constp,
            tc.tile_pool(name="persist", bufs=1) as persist,
            tc.tile_pool(name="xtp", bufs=12) as xtp,
            tc.tile_pool(name="uv", bufs=2) as uvp,
            tc.tile_pool(name="vt", bufs=2) as vtp,
            tc.tile_pool(name="ep", bufs=4) as ep,
            tc.tile_pool(name="zp", bufs=2) as zpool,
            tc.tile_pool(name="small", bufs=4) as smallp,
            tc.tile_pool(name="at", bufs=3) as atp,
            tc.tile_pool(name="rhs", bufs=2) as rhsp,
            tc.tile_pool(name="fout", bufs=2) as foutp,
            # PSUM (8 banks): psc 2 + ppv 2 (2 tags x1) + pzb 2 (2 tags x1),
            # plus pproj 2 while projections run; pproj closes before pwo
            # (2) opens.
            tc.tile_pool(name="psc", bufs=2, space="PSUM") as psc,
            tc.tile_pool(name="ppv", bufs=1, space="PSUM") as ppv,
            tc.tile_pool(name="pzb", bufs=1, space="PSUM") as pzb,
        ):
            # ---- constants ----
            cs1_sb = constp.tile([HD, S], BF)
            cs2_sb = constp.tile([HD, S], BF)
            mix1_sb = constp.tile([HD, HD], BF)
            mix2_sb = constp.tile([HD, HD], BF)
            onesc_sb = constp.tile([HD, 1], F32)
            onesr_sb = constp.tile([1, HD], F32)
            nc.gpsimd.dma_start(out=cs1_sb[:], in_=cs1[:])
            nc.gpsimd.dma_start(out=cs2_sb[:], in_=cs2[:])
            nc.gpsimd.dma_start(out=mix1_sb[:], in_=mix1[:])
            nc.gpsimd.dma_start(out=mix2_sb[:], in_=mix2[:])
            nc.gpsimd.dma_start(out=onesc_sb[:], in_=onesc[:])
            nc.gpsimd.dma_start(out=onesr_sb[:], in_=onesr[:])

            # ---- weights (wq and wo phases don't overlap: scoped pools) ----
            wk_sb = persist.tile([128, N_KC, HD], BF)
            wv_sb = persist.tile([128, N_KC, HD], BF)
            nc.sync.dma_start(out=wk_sb[:], in_=wkt[:])
            nc.sync.dma_start(out=wv_sb[:], in_=wvt[:])

            # ---- persistent activations ----
            qt_sb = persist.tile([128, N_QH, S], BF)
            kt_sb = persist.tile([128, S], BF)
            v_sb = persist.tile([128, N_KT, HD], BF)

            xt_tiles = {}   # (c, g) -> sbuf tile [128, 4, 512]

            def load_xt_group(c, g):
                t = xtp.tile([128, 4, 512], BF, name=f"xt{c}_{g}", tag="xt")
                nc.sync.dma_start(
                    out=t[:],
                    in_=xt[g * 4:(g + 1) * 4, :, c * 512:(c + 1) * 512].rearrange(
                        "g p n -> p g n"
                    ),
                )
                xt_tiles[(c, g)] = t

            for g in range(8):
                load_xt_group(0, g)

            # rope stage 2 (PE mix matmuls + copy) is deferred and flushed
            # mid-next-sweep, so the PE never waits on the DVE muls.
            pending_rope = []

            def flush_rope(pp):
                while pending_rope:
                    u, v, dst, key = pending_rope.pop(0)
                    rps = pp.tile([128, 512], F32, name=f"rps{key}", tag="pacc")
                    nc.tensor.matmul(rps[:], mix1_sb[:], u[:], start=True, stop=False)
                    nc.tensor.matmul(rps[:], mix2_sb[:], v[:], start=False, stop=True)
                    nc.scalar.copy(dst, rps[:])

            def rope_stage1(acc, dst, t0, key):
                """acc: [128,512] f32 PSUM -> u,v bf16; stage2 queued."""
                u = uvp.tile([128, 512], BF, name=f"u{key}", tag="u")
                v = uvp.tile([128, 512], BF, name=f"v{key}", tag="v")
                nc.vector.tensor_mul(u[:], acc[:], cs1_sb[:, t0:t0 + 512])
                nc.vector.tensor_mul(v[:], acc[:], cs2_sb[:, t0:t0 + 512])
                pending_rope.append((u, v, dst, key))

            def proj_drain(acc, c, kind, h):
                t0 = c * 512
                if kind == "q":
                    rope_stage1(acc, qt_sb[:, h, t0:t0 + 512], t0, f"q{c}_{h}")
                elif kind == "k":
                    rope_stage1(acc, kt_sb[:, t0:t0 + 512], t0, f"k{c}")
                else:
                    vt_t = vtp.tile([128, 512], BF, name=f"vt{c}", tag="vt")
                    nc.scalar.copy(vt_t[:], acc[:])
                    for g in range(4):
                        nc.sync.dma_start_transpose(
                            out=v_sb[:, c * 4 + g, :],
                            in_=vt_t[:, g * 128:(g + 1) * 128],
                        )

            def proj_sweep(pp, c, kind, h=0):
                """One staggered projection sweep: 32 MMs into one PSUM bank."""
                acc = pp.tile([128, 512], F32, name=f"acc_{kind}{c}_{h}", tag="pacc")
                if kind == "q":
                    wslice = lambda kc: wq_sb[:, kc, h * 128:(h + 1) * 128]
                elif kind == "k":
                    wslice = lambda kc: wk_sb[:, kc, :]
                else:
                    wslice = lambda kc: wv_sb[:, kc, :]
                for kc in range(N_KC):
                    if kc == 8:
                        flush_rope(pp)
                    nc.tensor.matmul(
                        acc[:], wslice(kc), xt_tiles[(c, kc // 4)][:, kc % 4, :],
                        start=(kc == 0), stop=(kc == N_KC - 1),
                    )
                proj_drain(acc, c, kind, h)

            def proj_sweep_gen(pp, c, kind, h=0):
                """proj_sweep as a filler generator: 16 yields of 2 MMs."""
                acc = pp.tile([128, 512], F32, name=f"acc_{kind}{c}_{h}", tag="pacc")
                if kind == "q":
                    wslice = lambda kc: wq_sb[:, kc, h * 128:(h + 1) * 128]
                elif kind == "k":
                    wslice = lambda kc: wk_sb[:, kc, :]
                else:
                    wslice = lambda kc: wv_sb[:, kc, :]
                for kp in range(16):
                    if kp == 4:
                        flush_rope(pp)
                    for kc in (2 * kp, 2 * kp + 1):
                        nc.tensor.matmul(
                            acc[:], wslice(kc), xt_tiles[(c, kc // 4)][:, kc % 4, :],
                            start=(kc == 0), stop=(kc == N_KC - 1),
                        )
                    if kp < 15:
                        yield
                proj_drain(acc, c, kind, h)
                yield

            wo_state = {}

            def wo_quarter_gen(qc, wop, quarter):
                """wo GEMM for chunk qc, one quarter (32 MMs, 16 yields).
                quarter 0: load rhs wave0, fps01 += wave0
                quarter 1: load rhs wave1, fps01 += wave1, emit fout01
                quarter 2: fps23 += wave0
                quarter 3: fps23 += wave1, emit fout23"""
                q0 = qc * 512
                hp = quarter % 2
                qsp = quarter // 2
                if quarter < 2:
                    r = rhsp.tile(
                        [128, 16, 512], BF, name=f"rhs{qc}_{hp}", tag="rhs"
                    )
                    nc.scalar.dma_start(
                        out=r[:],
                        in_=ag_out[qc][hp].rearrange("(t p) n -> p t n", p=128),
                    )
                    wo_state[(qc, hp)] = r
                rhs = wo_state[(qc, hp)]
                if hp == 0:
                    fps = [
                        wop.tile(
                            [128, 512], F32, name=f"f{qc}_{qsp}_{j}", tag=f"f{j}"
                        )
                        for j in range(2)
                    ]
                    wo_state[(qc, "fps", qsp)] = fps
                fps = wo_state[(qc, "fps", qsp)]
                for i in range(16):
                    ci, jj = i // 2, i % 2
                    hk = 4 * ci + 2 * hp + jj
                    for j in range(2):
                        qs = qsp * 2 + j
                        nc.tensor.matmul(
                            fps[j][:],
                            rhs[:, i, qs * 128:(qs + 1) * 128],
                            wo_sb[:, hk, :],
                            start=(hp == 0 and i == 0),
                            stop=(hp == 1 and i == 15),
                        )
                    if i < 15:
                        yield
                if hp == 1:
                    for j in range(2):
                        qs = qsp * 2 + j
                        f_sb = foutp.tile(
                            [128, 512], F32, name=f"fs{qc}_{qsp}_{j}", tag="fs"
                        )
                        nc.scalar.copy(f_sb[:], fps[j][:])
                        nc.sync.dma_start(
                            out=out_ext[q0 + qs * 128:q0 + (qs + 1) * 128, :],
                            in_=f_sb[:],
                        )
                yield

            def run_filler(f, n=1):
                if f is None:
                    return
                for _ in range(n):
                    try:
                        next(f)
                    except StopIteration:
                        break

            def attn_head(qc, h, filler=None):
                """Scores+exp+z+PV+normalize for one head of one q-chunk.
                Emits one filler slot (2 PE MMs) per k-tile to cover the
                exp-paced gaps in the in-order PE queue."""
                q0 = qc * 512
                pv = ppv.tile([128, 512], F32, name=f"pv{qc}_{h}", tag=f"pv{h % 2}")
                z_acc = zpool.tile([128, 512], F32, name=f"z{qc}_{h}", tag="zacc")
                stps = {}
                for kt in range(N_KT):
                    stp = psc.tile([128, 512], F32, name=f"st{qc}_{h}_{kt}", tag="st")
                    stps[kt] = stp
                    nc.tensor.matmul(
                        stp[:],
                        kt_sb[:, kt * 128:(kt + 1) * 128],
                        qt_sb[:, h, q0:q0 + 512],
                        start=True, stop=True,
                    )
                    run_filler(filler)
                    # process kt-1 (stay one score ahead of exp/PV)
                    if kt > 0:
                        emit_ev(qc, h, kt - 1, stps, pv, z_acc)
                emit_ev(qc, h, N_KT - 1, stps, pv, z_acc)
                run_filler(filler)
                # z partition-reduce, reciprocal, broadcast, normalize
                zr = pzb.tile([1, 512], F32, name=f"zr{qc}_{h}", tag="zr")
                nc.tensor.matmul(zr[:], onesc_sb[:], z_acc[:], start=True, stop=True)
                invz = smallp.tile([1, 512], F32, name=f"iz{qc}_{h}", tag="iz")
                nc.vector.reciprocal(invz[:], zr[:])
                bc = pzb.tile([128, 512], F32, name=f"bc{qc}_{h}", tag="bc")
                nc.tensor.matmul(bc[:], onesr_sb[:], invz[:], start=True, stop=True)
                bc_sb = smallp.tile([128, 512], BF, name=f"bcs{qc}_{h}", tag="bcs")
                nc.scalar.copy(bc_sb[:], bc[:])
                at_sb = atp.tile([128, 512], BF, name=f"at{qc}_{h}", tag="at")
                nc.vector.tensor_mul(at_sb[:], pv[:], bc_sb[:])
                hp, j = h // 2, h % 2
                nc.gpsimd.dma_start(
                    out=ag_in[qc][hp][j * 128:(j + 1) * 128, :], in_=at_sb[:]
                )

            def emit_ev(qc, h, kt, stps, pv, z_acc):
                e_t = ep.tile([128, 512], BF, name=f"e{qc}_{h}_{kt}", tag="e")
                nc.scalar.activation(
                    out=e_t[:], in_=stps[kt][:],
                    func=mybir.ActivationFunctionType.Exp,
                )
                nc.tensor.matmul(
                    pv[:], v_sb[:, kt, :], e_t[:],
                    start=(kt == 0), stop=(kt == N_KT - 1),
                )
                if kt == 0:
                    nc.vector.tensor_copy(z_acc[:], e_t[:])
                else:
                    nc.vector.tensor_add(z_acc[:], z_acc[:], e_t[:])

            def ag_launch(qc, hp):
                nc.gpsimd.collective_compute(
                    "AllGather",
                    mybir.AluOpType.bypass,
                    replica_groups=[list(range(N_CORES))],
                    ins=[ag_in[qc][hp][:].opt()],
                    outs=[ag_out[qc][hp][:].opt()],
                )

            with (
                tc.tile_pool(name="wqp", bufs=1) as wqp,
                tc.tile_pool(name="pproj", bufs=2, space="PSUM") as pproj,
            ):
                wq_sb = wqp.tile([128, N_KC, 512], BF)
                for ch in range(4):
                    nc.sync.dma_start(
                        out=wq_sb[:, ch * 8:(ch + 1) * 8, :],
                        in_=wqt[:, ch * 8:(ch + 1) * 8, :],
                    )
                for c in range(N_TC):
                    scope = nc.named_scope(f"proj{c}"); scope.__enter__()
                    if c + 1 < N_TC:
                        load_xt_group(c + 1, 0)
                    proj_sweep(pproj, c, "k")
                    if c + 1 < N_TC:
                        load_xt_group(c + 1, 1)
                    proj_sweep(pproj, c, "v")
                    if c + 1 < N_TC:
                        load_xt_group(c + 1, 2)
                        load_xt_group(c + 1, 3)
                    nq = N_QH if c < N_TC - 1 else 0
                    for h in range(nq):
                        proj_sweep(pproj, c, "q", h)
                        if c + 1 < N_TC and 4 + h < 8:
                            load_xt_group(c + 1, 4 + h)
                    scope.__exit__(None, None, None)

                # attention chunk 0; c3 Q sweeps interleaved as PE fillers
                scope = nc.named_scope("attn0"); scope.__enter__()
                for h in range(N_QH):
                    attn_head(0, h, filler=proj_sweep_gen(pproj, 3, "q", h))
                    if h % 2 == 1:
                        ag_launch(0, h // 2)
                flush_rope(pproj)
                scope.__exit__(None, None, None)

            with (
                tc.tile_pool(name="wop", bufs=1) as wop_s,
                tc.tile_pool(name="pwo", bufs=1, space="PSUM") as pwo,
            ):
                wo_sb = wop_s.tile([128, N_KC, 512], BF)
                for ch in range(4):
                    nc.gpsimd.dma_start(
                        out=wo_sb[:, ch * 8:(ch + 1) * 8, :],
                        in_=wot[:, ch * 8:(ch + 1) * 8, :],
                    )
                # filler schedule: block qc consumes wo quarters one head-slot
                # late, so each AllGather has ~2 head-slots of latency slack.
                pending = []   # queue of wo quarter generators
                for qc in range(1, N_TC):
                    scope = nc.named_scope(f"attn{qc}"); scope.__enter__()
                    for q in range(3):
                        pending.append(wo_quarter_gen(qc - 1, pwo, q))
                    for h in range(N_QH):
                        f = pending.pop(0) if pending else None
                        attn_head(qc, h, filler=f)
                        run_filler(f, 16)   # exhaust any leftover slots
                        if h % 2 == 1:
                            ag_launch(qc, h // 2)
                    pending = pending  # quarter 3 of qc-1 enqueued next block
                    pending.append(wo_quarter_gen(qc - 1, pwo, 3))
                    scope.__exit__(None, None, None)
                scope = nc.named_scope("wo3"); scope.__enter__()
                for f in pending:
                    run_filler(f, 17)
                for q in range(4):
                    run_filler(wo_quarter_gen(N_TC - 1, pwo, q), 17)
                scope.__exit__(None, None, None)

    _split_multi_waits(nc)
    return nc


def _host_prep(x, cos, sin, wq, wk, wv, wo):
    scale = np.float32(HD ** -0.5)
    perm = np.concatenate([np.arange(0, HD, 2), np.arange(1, HD, 2)])

    xt = np.ascontiguousarray(x.T.reshape(N_KC, 128, S)).astype(_BF16)
    cosT = cos.T.astype(np.float32)
    sinT = sin.T.astype(np.float32)
    cs1 = np.concatenate([cosT, sinT], axis=0).astype(_BF16)
    cs2 = np.concatenate([sinT, cosT], axis=0).astype(_BF16)

    m1 = np.zeros((HD, HD), np.float32)
    m1[np.arange(64), np.arange(64)] = 1.0
    m1[np.arange(64) + 64, np.arange(64)] = -1.0
    m2 = np.zeros((HD, HD), np.float32)
    m2[np.arange(64), np.arange(64) + 64] = 1.0
    m2[np.arange(64) + 64, np.arange(64) + 64] = 1.0

    def to_tiles(wT, ncols):
        # wT: [D, ncols] -> [128, N_KC, ncols] with [p, kc, n] = wT[kc*128+p, n]
        return np.ascontiguousarray(
            wT.reshape(N_KC, 128, ncols).transpose(1, 0, 2)
        ).astype(_BF16)

    shared = {
        "xt": xt,
        "cs1": cs1,
        "cs2": cs2,
        "mix1": m1.astype(_BF16),
        "mix2": m2.astype(_BF16),
        "onesc": np.ones((HD, 1), np.float32),
        "onesr": np.ones((1, HD), np.float32),
    }
    in_maps = []
    for c in range(N_CORES):
        wq_c = wq[c * 512:(c + 1) * 512].reshape(N_QH, HD, D)[:, perm, :]
        wq_c = wq_c.reshape(512, D) * scale
        wk_c = wk[c * HD:(c + 1) * HD][perm, :]
        wv_c = wv[c * HD:(c + 1) * HD]
        wo_c = wo[c * 512:(c + 1) * 512]
        m = dict(shared)
        m["wqt"] = to_tiles(np.ascontiguousarray(wq_c.T), 512)
        m["wkt"] = to_tiles(np.ascontiguousarray(wk_c.T), HD)
        m["wvt"] = to_tiles(np.ascontiguousarray(wv_c.T), HD)
        m["wot"] = to_tiles(np.ascontiguousarray(wo_c.T), 512)
        in_maps.append(m)
    return in_maps


def kernel(x, cos, sin, wq, wk, wv, wo, _trace=False):
    x = np.asarray(x, np.float32)
    cos = np.asarray(cos, np.float32)
    sin = np.asarray(sin, np.float32)
    wq = np.asarray(wq, np.float32)
    wk = np.asarray(wk, np.float32)
    wv = np.asarray(wv, np.float32)
    wo = np.asarray(wo, np.float32)

    in_maps = _host_prep(x, cos, sin, wq, wk, wv, wo)
    if "nc" not in _NC_CACHE:
        _NC_CACHE["nc"] = _build()
    nc = _NC_CACHE["nc"]
    res = run_bass_kernel_spmd(
        nc, in_maps, core_ids=list(range(N_CORES)), trace=_trace
    )
    out = np.concatenate([res.results[c]["out"] for c in range(N_CORES)], axis=1)
    out = np.ascontiguousarray(out, dtype=np.float32)
    if _trace:
        kernel._last_exec_time_ns = res.exec_time_ns
        kernel._last_result = res
    return out


# revision 23
# speedup vs baseline: 1.2091x; 1.0752x over previous
"""GQA attention (S=2048, D=4096, 32 Q heads / 8 KV heads, RoPE, full attn)
distributed over 8 Trainium2 NeuronCores.

Strategy (tensor-parallel by heads; AllGather of normalized attention before
the output projection, split per head-pair for pipelining):
  - core c owns Q heads 4c..4c+3 and KV head c (GQA groups align with cores).
  - all GEMMs in bf16 (stationary and moving), f32 PSUM accumulation.
  - projections as transposed GEMMs QT/KT/VT [chan, tok] via staggered
    per-output sweeps (one PSUM bank per output, 32 contraction MMs each),
    so banks release one at a time and RoPE drains overlap the next sweep.
  - RoPE on DVE only: u = p*cs1, v = p*cs2 (PSUM reads), then
    half-partition sub/add writes the bf16 QT/KT tiles. attn scale folded
    into wq host-side; channels deinterleaved (evens;odds) host-side.
  - V via a transposing SBUF->SBUF DMA instead of PE transposes.
  - attention per 512-token q-chunk, per head: scores ST=[k,q] per k-tile,
    exp on ScalarE ([128,512] -> bf16), z accumulated on DVE (running f32
    [128,512] + one ones-matmul partition reduce), PV accumulated in PSUM
    across all 16 k-tiles, normalize via DVE reciprocal + broadcast matmul.
  - the attention inner loop is ScalarE(exp)-paced, so independent "filler"
    matmuls (wo-GEMM quarters of earlier chunks, or the last chunk's Q
    projection sweeps) are interleaved between score/PV pairs to keep the
    in-order PE queue busy during exp waits.
  - AllGather per (chunk, head-pair): 8 collectives of [256,512]bf16 in,
    [2048,512]bf16 out; wo GEMM accumulates over the two gathered waves,
    2 PSUM banks (q-subtile pairs), quarters scheduled one head-slot late
    so each AG has ~2 head-slots of latency slack before first use.
Host side only reshapes/transposes/casts inputs and concatenates outputs.
"""
import sys

import numpy as np
import ml_dtypes

_BF16 = ml_dtypes.bfloat16

for _p in ("/root/.axon_site/_ro/trn_rl_repo", "/opt/trn_rl_repo"):
    if _p not in sys.path:
        sys.path.append(_p)

import concourse.bass as bass
import concourse.tile as tile
from concourse import mybir
from concourse.bass_utils import run_bass_kernel_spmd

N_CORES = 8
S = 2048
D = 4096
HD = 128
N_QH = 4          # Q heads per core
N_KT = S // 128   # 16 k-tiles
N_TC = S // 512   # 4 token chunks
N_KC = D // 128   # 32 contraction tiles
F32 = mybir.dt.float32
BF = mybir.dt.bfloat16

_NC_CACHE = {}


def _split_multi_waits(nc):
    """This container's walrus accepts only ONE sync-wait per instruction
    encoding; hoist extra waits onto fresh single-wait NoOps placed before
    the instruction on the same engine."""
    n = 0
    for fn in nc.m.functions:
        for bb in fn.blocks:
            new_insts = []
            changed = False
            for ins in bb.instructions:
                si = ins.sync_info
                waits = list(si.on_wait) if si is not None else []
                if len(waits) > 1:
                    for w in waits[:-1]:
                        n += 1
                        nop = mybir.InstNoOp(name=f"WSPL-{n}", ins=[], outs=[])
                        nop.engine = ins.engine
                        nop.sync_info = mybir.SyncInfo(on_wait=[w], on_update=[])
                        new_insts.append(nop)
                    si.on_wait = waits[-1:]
                    changed = True
                new_insts.append(ins)
            if changed:
                bb.instructions = new_insts
    return n


def _build():
    nc = bass.Bass()

    xt = nc.dram_tensor("xt", [N_KC, 128, S], BF, kind="ExternalInput")
    wqt = nc.dram_tensor("wqt", [128, N_KC, 512], BF, kind="ExternalInput")
    wkt = nc.dram_tensor("wkt", [128, N_KC, HD], BF, kind="ExternalInput")
    wvt = nc.dram_tensor("wvt", [128, N_KC, HD], BF, kind="ExternalInput")
    wot = nc.dram_tensor("wot", [128, N_KC, 512], BF, kind="ExternalInput")
    cs1 = nc.dram_tensor("cs1", [HD, S], BF, kind="ExternalInput")
    cs2 = nc.dram_tensor("cs2", [HD, S], BF, kind="ExternalInput")
    mix1 = nc.dram_tensor("mix1", [HD, HD], BF, kind="ExternalInput")
    mix2 = nc.dram_tensor("mix2", [HD, HD], BF, kind="ExternalInput")
    onesc = nc.dram_tensor("onesc", [HD, 1], F32, kind="ExternalInput")
    onesr = nc.dram_tensor("onesr", [1, HD], BF, kind="ExternalInput")
    out_ext = nc.dram_tensor("out", [S, 512], F32, kind="ExternalOutput")

    ag_in = [
        [nc.dram_tensor(f"agi{qc}_{hp}", [256, 512], BF) for hp in range(2)]
        for qc in range(N_TC)
    ]
    ag_out = [
        [
            nc.dram_tensor(f"ago{qc}_{hp}", [2048, 512], BF, addr_space="Shared")
            for hp in range(2)
        ]
        for qc in range(N_TC)
    ]

    with tile.TileContext(nc) as tc:
        with (
            tc.tile_pool(name="const", bufs=1) as constp,
            tc.tile_pool(name="persist", bufs=1) as persist,
            tc.tile_pool(name="xtp", bufs=12) as xtp,
            tc.tile_pool(name="uv", bufs=2) as uvp,
            tc.tile_pool(name="vt", bufs=2) as vtp,
            tc.tile_pool(name="ep", bufs=4) as ep,
            tc.tile_pool(name="zp", bufs=2) as zpool,
            tc.tile_pool(name="small", bufs=4) as smallp,
            tc.tile_pool(name="at", bufs=3) as atp,
            tc.tile_pool(name="rhs", bufs=2) as rhsp,
            tc.tile_pool(name="fout", bufs=2) as foutp,
            # PSUM (8 banks): psc 2 + ppv 2 (2 tags x1) + pzb 2 (2 tags x1),
            # plus pproj 2 while projections run; pproj closes before pwo
            # (2) opens.
            tc.tile_pool(name="psc", bufs=3, space="PSUM") as psc,
            tc.tile_pool(name="ppv", bufs=1, space="PSUM") as ppv,
            tc.tile_pool(name="pzb", bufs=1, space="PSUM") as pzb,
        ):
            # ---- constants ----
            cs1_sb = constp.tile([HD, S], BF)
            cs2_sb = constp.tile([HD, S], BF)
            mix1_sb = constp.tile([HD, HD], BF)
            mix2_sb = constp.tile([HD, HD], BF)
            onesc_sb = constp.tile([HD, 1], F32)
            onesr_sb = constp.tile([1, HD], BF)
            nc.gpsimd.dma_start(out=cs1_sb[:], in_=cs1[:])
            nc.gpsimd.dma_start(out=cs2_sb[:], in_=cs2[:])
            nc.gpsimd.dma_start(out=mix1_sb[:], in_=mix1[:])
            nc.gpsimd.dma_start(out=mix2_sb[:], in_=mix2[:])
            nc.gpsimd.dma_start(out=onesc_sb[:], in_=onesc[:])
            nc.gpsimd.dma_start(out=onesr_sb[:], in_=onesr[:])

            # ---- weights (wq and wo phases don't overlap: scoped pools) ----
            wk_sb = persist.tile([128, N_KC, HD], BF)
            wv_sb = persist.tile([128, N_KC, HD], BF)
            nc.sync.dma_start(out=wk_sb[:], in_=wkt[:])
            nc.sync.dma_start(out=wv_sb[:], in_=wvt[:])

            # ---- persistent activations ----
            qt_sb = persist.tile([128, N_QH, S], BF)
            kt_sb = persist.tile([128, S], BF)
            v_sb = persist.tile([128, N_KT, HD], BF)

            xt_tiles = {}   # (c, g) -> sbuf tile [128, 4, 512]

            def load_xt_group(c, g):
                t = xtp.tile([128, 4, 512], BF, name=f"xt{c}_{g}", tag="xt")
                nc.sync.dma_start(
                    out=t[:],
                    in_=xt[g * 4:(g + 1) * 4, :, c * 512:(c + 1) * 512].rearrange(
                        "g p n -> p g n"
                    ),
                )
                xt_tiles[(c, g)] = t

            for g in range(8):
                load_xt_group(0, g)

            # rope stage 2 (PE mix matmuls + copy) is deferred and flushed
            # mid-next-sweep, so the PE never waits on the DVE muls.
            pending_rope = []

            def flush_rope(pp):
                while pending_rope:
                    u, v, dst, key = pending_rope.pop(0)
                    rps = pp.tile([128, 512], F32, name=f"rps{key}", tag="pacc")
                    nc.tensor.matmul(rps[:], mix1_sb[:], u[:], start=True, stop=False)
                    nc.tensor.matmul(rps[:], mix2_sb[:], v[:], start=False, stop=True)
                    nc.scalar.copy(dst, rps[:])

            def rope_stage1(acc, dst, t0, key):
                """acc: [128,512] f32 PSUM -> u,v bf16; stage2 queued."""
                u = uvp.tile([128, 512], BF, name=f"u{key}", tag="u")
                v = uvp.tile([128, 512], BF, name=f"v{key}", tag="v")
                nc.vector.tensor_mul(u[:], acc[:], cs1_sb[:, t0:t0 + 512])
                nc.vector.tensor_mul(v[:], acc[:], cs2_sb[:, t0:t0 + 512])
                pending_rope.append((u, v, dst, key))

            def proj_drain(acc, c, kind, h):
                t0 = c * 512
                if kind == "q":
                    rope_stage1(acc, qt_sb[:, h, t0:t0 + 512], t0, f"q{c}_{h}")
                elif kind == "k":
                    rope_stage1(acc, kt_sb[:, t0:t0 + 512], t0, f"k{c}")
                else:
                    vt_t = vtp.tile([128, 512], BF, name=f"vt{c}", tag="vt")
                    nc.scalar.copy(vt_t[:], acc[:])
                    for g in range(4):
                        nc.sync.dma_start_transpose(
                            out=v_sb[:, c * 4 + g, :],
                            in_=vt_t[:, g * 128:(g + 1) * 128],
                        )

            def proj_sweep(pp, c, kind, h=0):
                """One staggered projection sweep: 32 MMs into one PSUM bank."""
                acc = pp.tile([128, 512], F32, name=f"acc_{kind}{c}_{h}", tag="pacc")
                if kind == "q":
                    wslice = lambda kc: wq_sb[:, kc, h * 128:(h + 1) * 128]
                elif kind == "k":
                    wslice = lambda kc: wk_sb[:, kc, :]
                else:
                    wslice = lambda kc: wv_sb[:, kc, :]
                for kc in range(N_KC):
                    if kc == 8:
                        flush_rope(pp)
                    nc.tensor.matmul(
                        acc[:], wslice(kc), xt_tiles[(c, kc // 4)][:, kc % 4, :],
                        start=(kc == 0), stop=(kc == N_KC - 1),
                    )
                proj_drain(acc, c, kind, h)

            def proj_sweep_gen(pp, c, kind, h=0):
                """proj_sweep as a filler generator: 16 yields of 2 MMs."""
                acc = pp.tile([128, 512], F32, name=f"acc_{kind}{c}_{h}", tag="pacc")
                if kind == "q":
                    wslice = lambda kc: wq_sb[:, kc, h * 128:(h + 1) * 128]
                elif kind == "k":
                    wslice = lambda kc: wk_sb[:, kc, :]
                else:
                    wslice = lambda kc: wv_sb[:, kc, :]
                for kp in range(16):
                    if kp == 4:
                        flush_rope(pp)
                    for kc in (2 * kp, 2 * kp + 1):
                        nc.tensor.matmul(
                            acc[:], wslice(kc), xt_tiles[(c, kc // 4)][:, kc % 4, :],
                            start=(kc == 0), stop=(kc == N_KC - 1),
                        )
                    if kp < 15:
                        yield
                proj_drain(acc, c, kind, h)
                yield

            wo_state = {}

            def wo_quarter_gen(qc, wop, quarter):
                """wo GEMM for chunk qc, one quarter (32 MMs, 16 yields).
                quarter 0: load rhs wave0, fps01 += wave0
                quarter 1: load rhs wave1, fps01 += wave1, emit fout01
                quarter 2: fps23 += wave0
                quarter 3: fps23 += wave1, emit fout23"""
                q0 = qc * 512
                hp = quarter % 2
                qsp = quarter // 2
                if quarter < 2:
                    r = rhsp.tile(
                        [128, 16, 512], BF, name=f"rhs{qc}_{hp}", tag="rhs"
                    )
                    nc.scalar.dma_start(
                        out=r[:],
                        in_=ag_out[qc][hp].rearrange("(t p) n -> p t n", p=128),
                    )
                    wo_state[(qc, hp)] = r
                rhs = wo_state[(qc, hp)]
                if hp == 0:
                    fps = [
                        wop.tile(
                            [128, 512], F32, name=f"f{qc}_{qsp}_{j}", tag=f"f{j}"
                        )
                        for j in range(2)
                    ]
                    wo_state[(qc, "fps", qsp)] = fps
                fps = wo_state[(qc, "fps", qsp)]
                for i in range(16):
                    ci, jj = i // 2, i % 2
                    hk = 4 * ci + 2 * hp + jj
                    for j in range(2):
                        qs = qsp * 2 + j
                        nc.tensor.matmul(
                            fps[j][:],
                            rhs[:, i, qs * 128:(qs + 1) * 128],
                            wo_sb[:, hk, :],
                            start=(hp == 0 and i == 0),
                            stop=(hp == 1 and i == 15),
                        )
                    if i < 15:
                        yield
                if hp == 1:
                    for j in range(2):
                        qs = qsp * 2 + j
                        f_sb = foutp.tile(
                            [128, 512], F32, name=f"fs{qc}_{qsp}_{j}", tag="fs"
                        )
                        nc.scalar.copy(f_sb[:], fps[j][:])
                        nc.sync.dma_start(
                            out=out_ext[q0 + qs * 128:q0 + (qs + 1) * 128, :],
                            in_=f_sb[:],
                        )
                yield

            def run_filler(f, n=1):
                if f is None:
                    return
                for _ in range(n):
                    try:
                        next(f)
                    except StopIteration:
                        break

            class NormTail:
                """Deferred per-head softmax normalization: the tiny zr/bc
                matmuls are emitted inside the NEXT head's kt stream so the
                in-order PE never waits on the DVE z-chain or ScalarE 1/z."""

                def __init__(self, qc, h, pv, z_acc):
                    self.qc, self.h, self.pv, self.z_acc = qc, h, pv, z_acc

                def stage_a(self):
                    qc, h = self.qc, self.h
                    zr = pzb.tile([1, 512], F32, name=f"zr{qc}_{h}", tag="zb")
                    nc.tensor.matmul(
                        zr[:], onesc_sb[:], self.z_acc[:], start=True, stop=True
                    )
                    lnz = smallp.tile([1, 512], F32, name=f"ln{qc}_{h}", tag="lnz")
                    nc.scalar.activation(
                        out=lnz[:], in_=zr[:],
                        func=mybir.ActivationFunctionType.Ln,
                    )
                    self.invz = smallp.tile([1, 512], BF, name=f"iz{qc}_{h}", tag="iz")
                    nc.scalar.activation(
                        out=self.invz[:], in_=lnz[:],
                        func=mybir.ActivationFunctionType.Exp, scale=-1.0,
                    )

                def stage_b(self):
                    qc, h = self.qc, self.h
                    bc = pzb.tile([128, 512], F32, name=f"bc{qc}_{h}", tag="zb")
                    nc.tensor.matmul(
                        bc[:], onesr_sb[:], self.invz[:], start=True, stop=True
                    )
                    bc_sb = smallp.tile([128, 512], BF, name=f"bcs{qc}_{h}", tag="bcs")
                    nc.scalar.copy(bc_sb[:], bc[:])
                    at_sb = atp.tile([128, 512], BF, name=f"at{qc}_{h}", tag="at")
                    nc.vector.tensor_mul(at_sb[:], self.pv[:], bc_sb[:])
                    hp, j = h // 2, h % 2
                    nc.gpsimd.dma_start(
                        out=ag_in[qc][hp][j * 128:(j + 1) * 128, :], in_=at_sb[:]
                    )
                    if j == 1:
                        ag_launch(qc, hp)

            def attn_head(qc, h, filler=None, prev_tail=None):
                """Scores+exp+z+PV for one head of one q-chunk. Emits one
                filler slot (2 PE MMs) per k-tile to cover the exp-paced
                gaps in the in-order PE queue; the previous head's normalize
                tail is emitted at kt 2 and 8. Returns this head's tail."""
                q0 = qc * 512
                pv = ppv.tile([128, 512], F32, name=f"pv{qc}_{h}", tag=f"pv{h % 2}")
                z_acc = zpool.tile([128, 512], F32, name=f"z{qc}_{h}", tag="zacc")
                stps = {}
                for kt in range(N_KT):
                    if prev_tail is not None:
                        if kt == 2:
                            prev_tail.stage_a()
                        elif kt == 8:
                            prev_tail.stage_b()
                    stp = psc.tile([128, 512], F32, name=f"st{qc}_{h}_{kt}", tag="st")
                    stps[kt] = stp
                    nc.tensor.matmul(
                        stp[:],
                        kt_sb[:, kt * 128:(kt + 1) * 128],
                        qt_sb[:, h, q0:q0 + 512],
                        start=True, stop=True,
                    )
                    run_filler(filler)
                    # process kt-1 (stay one score ahead of exp/PV)
                    if kt > 0:
                        emit_ev(qc, h, kt - 1, stps, pv, z_acc)
                emit_ev(qc, h, N_KT - 1, stps, pv, z_acc)
                run_filler(filler)
                return NormTail(qc, h, pv, z_acc)

            def emit_ev(qc, h, kt, stps, pv, z_acc):
                e_t = ep.tile([128, 512], BF, name=f"e{qc}_{h}_{kt}", tag="e")
                nc.scalar.activation(
                    out=e_t[:], in_=stps[kt][:],
                    func=mybir.ActivationFunctionType.Exp,
                )
                nc.tensor.matmul(
                    pv[:], v_sb[:, kt, :], e_t[:],
                    start=(kt == 0), stop=(kt == N_KT - 1),
                )
                if kt == 0:
                    nc.vector.tensor_copy(z_acc[:], e_t[:])
                else:
                    nc.vector.tensor_add(z_acc[:], z_acc[:], e_t[:])

            def ag_launch(qc, hp):
                nc.gpsimd.collective_compute(
                    "AllGather",
                    mybir.AluOpType.bypass,
                    replica_groups=[list(range(N_CORES))],
                    ins=[ag_in[qc][hp][:].opt()],
                    outs=[ag_out[qc][hp][:].opt()],
                )

            with (
                tc.tile_pool(name="wqp", bufs=1) as wqp,
                tc.tile_pool(name="pproj", bufs=2, space="PSUM") as pproj,
            ):
                wq_sb = wqp.tile([128, N_KC, 512], BF)
                for ch in range(4):
                    nc.sync.dma_start(
                        out=wq_sb[:, ch * 8:(ch + 1) * 8, :],
                        in_=wqt[:, ch * 8:(ch + 1) * 8, :],
                    )
                for c in range(N_TC):
                    scope = nc.named_scope(f"proj{c}"); scope.__enter__()
                    if c + 1 < N_TC:
                        load_xt_group(c + 1, 0)
                    proj_sweep(pproj, c, "k")
                    if c + 1 < N_TC:
                        load_xt_group(c + 1, 1)
                    proj_sweep(pproj, c, "v")
                    if c + 1 < N_TC:
                        load_xt_group(c + 1, 2)
                        load_xt_group(c + 1, 3)
                    nq = N_QH if c < N_TC - 1 else 0
                    for h in range(nq):
                        proj_sweep(pproj, c, "q", h)
                        if c + 1 < N_TC and 4 + h < 8:
                            load_xt_group(c + 1, 4 + h)
                    scope.__exit__(None, None, None)

                # attention chunk 0; c3 Q sweeps interleaved as PE fillers
                scope = nc.named_scope("attn0"); scope.__enter__()
                tail = None
                for h in range(N_QH):
                    tail = attn_head(
                        0, h, filler=proj_sweep_gen(pproj, 3, "q", h),
                        prev_tail=tail,
                    )
                flush_rope(pproj)
                scope.__exit__(None, None, None)

            with (
                tc.tile_pool(name="wop", bufs=1) as wop_s,
                tc.tile_pool(name="pwo", bufs=1, space="PSUM") as pwo,
            ):
                wo_sb = wop_s.tile([128, N_KC, 512], BF)
                for ch in range(4):
                    nc.gpsimd.dma_start(
                        out=wo_sb[:, ch * 8:(ch + 1) * 8, :],
                        in_=wot[:, ch * 8:(ch + 1) * 8, :],
                    )
                # filler schedule: block qc consumes wo quarters one head-slot
                # late, so each AllGather has ~2 head-slots of latency slack.
                pending = [None]   # queue of wo quarter generators
                for qc in range(1, N_TC):
                    scope = nc.named_scope(f"attn{qc}"); scope.__enter__()
                    for q in range(3):
                        pending.append(wo_quarter_gen(qc - 1, pwo, q))
                    for h in range(N_QH):
                        f = pending.pop(0) if pending else None
                        tail = attn_head(qc, h, filler=f, prev_tail=tail)
                        run_filler(f, 16)   # exhaust any leftover slots
                    pending.append(wo_quarter_gen(qc - 1, pwo, 3))
                    scope.__exit__(None, None, None)
                scope = nc.named_scope("wo3"); scope.__enter__()
                # flush the last attention head's normalize before the tail
                tail.stage_a()
                tail.stage_b()
                for f in pending:
                    run_filler(f, 17)
                for q in range(4):
                    run_filler(wo_quarter_gen(N_TC - 1, pwo, q), 17)
                scope.__exit__(None, None, None)

    _split_multi_waits(nc)
    return nc


def _host_prep(x, cos, sin, wq, wk, wv, wo):
    scale = np.float32(HD ** -0.5)
    perm = np.concatenate([np.arange(0, HD, 2), np.arange(1, HD, 2)])

    xt = np.ascontiguousarray(x.T.reshape(N_KC, 128, S)).astype(_BF16)
    cosT = cos.T.astype(np.float32)
    sinT = sin.T.astype(np.float32)
    cs1 = np.concatenate([cosT, sinT], axis=0).astype(_BF16)
    cs2 = np.concatenate([sinT, cosT], axis=0).astype(_BF16)

    m1 = np.zeros((HD, HD), np.float32)
    m1[np.arange(64), np.arange(64)] = 1.0
    m1[np.arange(64) + 64, np.arange(64)] = -1.0
    m2 = np.zeros((HD, HD), np.float32)
    m2[np.arange(64), np.arange(64) + 64] = 1.0
    m2[np.arange(64) + 64, np.arange(64) + 64] = 1.0

    def to_tiles(wT, ncols):
        # wT: [D, ncols] -> [128, N_KC, ncols] with [p, kc, n] = wT[kc*128+p, n]
        return np.ascontiguousarray(
            wT.reshape(N_KC, 128, ncols).transpose(1, 0, 2)
        ).astype(_BF16)

    shared = {
        "xt": xt,
        "cs1": cs1,
        "cs2": cs2,
        "mix1": m1.astype(_BF16),
        "mix2": m2.astype(_BF16),
        "onesc": np.ones((HD, 1), np.float32),
        "onesr": np.ones((1, HD), np.float32).astype(_BF16),
    }
    in_maps = []
    for c in range(N_CORES):
        wq_c = wq[c * 512:(c + 1) * 512].reshape(N_QH, HD, D)[:, perm, :]
        wq_c = wq_c.reshape(512, D) * scale
        wk_c = wk[c * HD:(c + 1) * HD][perm, :]
        wv_c = wv[c * HD:(c + 1) * HD]
        wo_c = wo[c * 512:(c + 1) * 512]
        m = dict(shared)
        m["wqt"] = to_tiles(np.ascontiguousarray(wq_c.T), 512)
        m["wkt"] = to_tiles(np.ascontiguousarray(wk_c.T), HD)
        m["wvt"] = to_tiles(np.ascontiguousarray(wv_c.T), HD)
        m["wot"] = to_tiles(np.ascontiguousarray(wo_c.T), 512)
        in_maps.append(m)
    return in_maps


def kernel(x, cos, sin, wq, wk, wv, wo, _trace=False):
    x = np.asarray(x, np.float32)
    cos = np.asarray(cos, np.float32)
    sin = np.asarray(sin, np.float32)
    wq = np.asarray(wq, np.float32)
    wk = np.asarray(wk, np.float32)
    wv = np.asarray(wv, np.float32)
    wo = np.asarray(wo, np.float32)

    in_maps = _host_prep(x, cos, sin, wq, wk, wv, wo)
    if "nc" not in _NC_CACHE:
        _NC_CACHE["nc"] = _build()
    nc = _NC_CACHE["nc"]
    res = run_bass_kernel_spmd(
        nc, in_maps, core_ids=list(range(N_CORES)), trace=_trace
    )
    out = np.concatenate([res.results[c]["out"] for c in range(N_CORES)], axis=1)
    out = np.ascontiguousarray(out, dtype=np.float32)
    if _trace:
        kernel._last_exec_time_ns = res.exec_time_ns
        kernel._last_result = res
    return out


# revision 28
# speedup vs baseline: 1.2356x; 1.0219x over previous
"""GQA attention (S=2048, D=4096, 32 Q heads / 8 KV heads, RoPE, full attn)
distributed over 8 Trainium2 NeuronCores.

Strategy (tensor-parallel by heads; AllGather of normalized attention before
the output projection):
  - core c owns Q heads 4c..4c+3 and KV head c (GQA groups align with cores).
  - all GEMMs bf16 (stationary and moving), f32 PSUM accumulation.
  - projections as transposed GEMMs QT/KT/VT [chan, tok]: sweeps PAIRED so
    consecutive matmuls alternate between two PSUM banks (single-bank
    back-to-back accumulation loses ~70ns/MM of drain overlap); chunks 0-2
    use a dedicated 4-bank ring, chunk 3's K/V + Q sweeps share a 2-bank
    ring with the attention-chunk-0 fillers.
  - RoPE via DVE muls (u=p*cs1, v=p*cs2) + deferred PE mix-matmuls flushed
    mid-next-sweep (never blocks the PE on the DVE); V transposed by
    SBUF->SBUF transposing DMAs.
  - attention per 512-token q-chunk, per head: scores ST=[k,q] per k-tile
    (3-bank ring), exp on ScalarE -> bf16, z on DVE (running f32 sum +
    ones-matmul partition reduce), PV accumulated over 16 k-tiles;
    normalize (1/z via ScalarE ln/exp, ones-broadcast matmul, DVE mul)
    deferred into the NEXT head's kt stream so the PE never waits.
  - the attention inner loop is exp-paced, so independent filler matmuls
    (wo-GEMM quarters of earlier chunks / last chunk's Q sweeps) fill the
    in-order PE queue between score/PV pairs.
  - AllGather per (chunk, head-pair) for chunks 0-2 ([256,512]bf16 ->
    [2048,512]) and per HEAD for chunk 3 ([128,512] -> [1024,512]) so the
    tail wo GEMM never waits; gathered tiles are DMA'd in half-waves
    pre-issued as soon as each collective is launched.
Host side only reshapes/transposes/casts inputs and concatenates outputs.
"""
import sys

import numpy as np
import ml_dtypes

_BF16 = ml_dtypes.bfloat16

for _p in ("/root/.axon_site/_ro/trn_rl_repo", "/opt/trn_rl_repo"):
    if _p not in sys.path:
        sys.path.append(_p)

import concourse.bass as bass
import concourse.tile as tile
from concourse import mybir
from concourse.bass_utils import run_bass_kernel_spmd

N_CORES = 8
S = 2048
D = 4096
HD = 128
N_QH = 4          # Q heads per core
N_KT = S // 128   # 16 k-tiles
N_TC = S // 512   # 4 token chunks
N_KC = D // 128   # 32 contraction tiles
F32 = mybir.dt.float32
BF = mybir.dt.bfloat16

_NC_CACHE = {}


def _split_multi_waits(nc):
    """This container's walrus accepts only ONE sync-wait per instruction
    encoding; hoist extra waits onto fresh single-wait NoOps placed before
    the instruction on the same engine."""
    n = 0
    for fn in nc.m.functions:
        for bb in fn.blocks:
            new_insts = []
            changed = False
            for ins in bb.instructions:
                si = ins.sync_info
                waits = list(si.on_wait) if si is not None else []
                if len(waits) > 1:
                    for w in waits[:-1]:
                        n += 1
                        nop = mybir.InstNoOp(name=f"WSPL-{n}", ins=[], outs=[])
                        nop.engine = ins.engine
                        nop.sync_info = mybir.SyncInfo(on_wait=[w], on_update=[])
                        new_insts.append(nop)
                    si.on_wait = waits[-1:]
                    changed = True
                new_insts.append(ins)
            if changed:
                bb.instructions = new_insts
    return n


def _build():
    nc = bass.Bass()

    xt = nc.dram_tensor("xt", [N_KC, 128, S], BF, kind="ExternalInput")
    wqt = nc.dram_tensor("wqt", [128, N_KC, 512], BF, kind="ExternalInput")
    wkt = nc.dram_tensor("wkt", [128, N_KC, HD], BF, kind="ExternalInput")
    wvt = nc.dram_tensor("wvt", [128, N_KC, HD], BF, kind="ExternalInput")
    wot = nc.dram_tensor("wot", [128, N_KC, 512], BF, kind="ExternalInput")
    cs1 = nc.dram_tensor("cs1", [HD, S], BF, kind="ExternalInput")
    cs2 = nc.dram_tensor("cs2", [HD, S], BF, kind="ExternalInput")
    mix1 = nc.dram_tensor("mix1", [HD, HD], BF, kind="ExternalInput")
    mix2 = nc.dram_tensor("mix2", [HD, HD], BF, kind="ExternalInput")
    onesc = nc.dram_tensor("onesc", [HD, 1], F32, kind="ExternalInput")
    onesr = nc.dram_tensor("onesr", [1, HD], BF, kind="ExternalInput")
    out_ext = nc.dram_tensor("out", [S, 512], F32, kind="ExternalOutput")

    # chunks 0-2: AllGather per head-pair; chunk 3: per head
    ag_in = {}
    ag_out = {}
    for qc in range(3):
        for hp in range(2):
            ag_in[(qc, hp)] = nc.dram_tensor(f"agi{qc}_{hp}", [256, 512], BF)
            ag_out[(qc, hp)] = nc.dram_tensor(
                f"ago{qc}_{hp}", [2048, 512], BF, addr_space="Shared"
            )
    for h in range(N_QH):
        ag_in[(3, h)] = nc.dram_tensor(f"agi3_{h}", [128, 512], BF)
        ag_out[(3, h)] = nc.dram_tensor(
            f"ago3_{h}", [1024, 512], BF, addr_space="Shared"
        )

    with tile.TileContext(nc) as tc:
        with (
            tc.tile_pool(name="const", bufs=1) as constp,
            tc.tile_pool(name="persist", bufs=1) as persist,
            tc.tile_pool(name="xtp", bufs=12) as xtp,
            tc.tile_pool(name="uv", bufs=2) as uvp,
            tc.tile_pool(name="vt", bufs=2) as vtp,
            tc.tile_pool(name="ep", bufs=4) as ep,
            tc.tile_pool(name="zp", bufs=2) as zpool,
            tc.tile_pool(name="small", bufs=4) as smallp,
            tc.tile_pool(name="at", bufs=3) as atp,
            tc.tile_pool(name="rhs", bufs=5) as rhsp,
            tc.tile_pool(name="fout", bufs=2) as foutp,
        ):
            # ---- constants ----
            cs1_sb = constp.tile([HD, S], BF)
            cs2_sb = constp.tile([HD, S], BF)
            mix1_sb = constp.tile([HD, HD], BF)
            mix2_sb = constp.tile([HD, HD], BF)
            onesc_sb = constp.tile([HD, 1], F32)
            onesr_sb = constp.tile([1, HD], BF)
            nc.gpsimd.dma_start(out=cs1_sb[:], in_=cs1[:])
            nc.gpsimd.dma_start(out=cs2_sb[:], in_=cs2[:])
            nc.gpsimd.dma_start(out=mix1_sb[:], in_=mix1[:])
            nc.gpsimd.dma_start(out=mix2_sb[:], in_=mix2[:])
            nc.gpsimd.dma_start(out=onesc_sb[:], in_=onesc[:])
            nc.gpsimd.dma_start(out=onesr_sb[:], in_=onesr[:])

            # ---- weights (wq and wo phases don't overlap: scoped pools) ----
            wk_sb = persist.tile([128, N_KC, HD], BF)
            wv_sb = persist.tile([128, N_KC, HD], BF)
            nc.sync.dma_start(out=wk_sb[:], in_=wkt[:])
            nc.sync.dma_start(out=wv_sb[:], in_=wvt[:])

            # ---- persistent activations ----
            qt_sb = persist.tile([128, N_QH, S], BF)
            kt_sb = persist.tile([128, S], BF)
            v_sb = persist.tile([128, N_KT, HD], BF)

            xt_tiles = {}   # (c, g) -> sbuf tile [128, 4, 512]

            def load_xt_group(c, g):
                t = xtp.tile([128, 4, 512], BF, name=f"xt{c}_{g}", tag="xt")
                nc.sync.dma_start(
                    out=t[:],
                    in_=xt[g * 4:(g + 1) * 4, :, c * 512:(c + 1) * 512].rearrange(
                        "g p n -> p g n"
                    ),
                )
                xt_tiles[(c, g)] = t

            for g in range(8):
                load_xt_group(0, g)

            # rope stage 2 (PE mix matmuls + copy) is deferred and flushed
            # mid-next-sweep, so the PE never waits on the DVE muls.
            pending_rope = []

            def flush_rope(pool):
                while pending_rope:
                    u, v, dst, key = pending_rope.pop(0)
                    rps = pool.tile([128, 512], F32, name=f"rps{key}", tag=pool._ropetag)
                    nc.tensor.matmul(rps[:], mix1_sb[:], u[:], start=True, stop=False)
                    nc.tensor.matmul(rps[:], mix2_sb[:], v[:], start=False, stop=True)
                    nc.scalar.copy(dst, rps[:])

            def rope_stage1(acc, dst, t0, key):
                u = uvp.tile([128, 512], BF, name=f"u{key}", tag="u")
                v = uvp.tile([128, 512], BF, name=f"v{key}", tag="v")
                nc.vector.tensor_mul(u[:], acc[:], cs1_sb[:, t0:t0 + 512])
                nc.vector.tensor_mul(v[:], acc[:], cs2_sb[:, t0:t0 + 512])
                pending_rope.append((u, v, dst, key))

            def proj_drain(acc, c, kind, h):
                t0 = c * 512
                if kind == "q":
                    rope_stage1(acc, qt_sb[:, h, t0:t0 + 512], t0, f"q{c}_{h}")
                elif kind == "k":
                    rope_stage1(acc, kt_sb[:, t0:t0 + 512], t0, f"k{c}")
                else:
                    vt_t = vtp.tile([128, 512], BF, name=f"vt{c}", tag="vt")
                    nc.scalar.copy(vt_t[:], acc[:])
                    for g in range(4):
                        nc.sync.dma_start_transpose(
                            out=v_sb[:, c * 4 + g, :],
                            in_=vt_t[:, g * 128:(g + 1) * 128],
                        )

            def wslice_fn(kind, h):
                if kind == "q":
                    return lambda kc: wq_sb[:, kc, h * 128:(h + 1) * 128]
                if kind == "k":
                    return lambda kc: wk_sb[:, kc, :]
                return lambda kc: wv_sb[:, kc, :]

            def proj_pair(pp, rope_pool, c, specs):
                """Two interleaved 32-MM sweeps (bank-alternating)."""
                accs = [
                    (
                        pp.tile([128, 512], F32, name=f"acc_{k}{c}_{h}", tag="pacc"),
                        wslice_fn(k, h), k, h,
                    )
                    for k, h in specs
                ]
                for kc in range(N_KC):
                    if kc == 8:
                        flush_rope(rope_pool)
                    for acc, ws, _, _ in accs:
                        nc.tensor.matmul(
                            acc[:], ws(kc), xt_tiles[(c, kc // 4)][:, kc % 4, :],
                            start=(kc == 0), stop=(kc == N_KC - 1),
                        )
                for acc, _, k, h in accs:
                    proj_drain(acc, c, k, h)

            def proj_pair_gen(pp, rope_pool, c, specs):
                """proj_pair as a filler generator: 32 yields of 2 MMs."""
                accs = [
                    (
                        pp.tile([128, 512], F32, name=f"acc_{k}{c}_{h}", tag="pacc"),
                        wslice_fn(k, h), k, h,
                    )
                    for k, h in specs
                ]
                for kc in range(N_KC):
                    if kc == 8:
                        flush_rope(rope_pool)
                    for acc, ws, _, _ in accs:
                        nc.tensor.matmul(
                            acc[:], ws(kc), xt_tiles[(c, kc // 4)][:, kc % 4, :],
                            start=(kc == 0), stop=(kc == N_KC - 1),
                        )
                    if kc < N_KC - 1:
                        yield
                for acc, _, k, h in accs:
                    proj_drain(acc, c, k, h)
                yield

            # ---------- wo GEMM machinery ----------
            wo_state = {}
            cur_pools = {}   # phase-scoped PSUM pools for attention

            def load_rhs_halves(qc, hp):
                """DMA one gathered wave into two [128,8,512] half tiles."""
                halves = []
                for half in range(2):
                    r = rhsp.tile(
                        [128, 8, 512], BF, name=f"rhs{qc}_{hp}_{half}", tag="rhs"
                    )
                    nc.scalar.dma_start(
                        out=r[:],
                        in_=ag_out[(qc, hp)][half * 1024:(half + 1) * 1024, :]
                        .rearrange("(t p) n -> p t n", p=128),
                    )
                    halves.append(r)
                wo_state[(qc, hp)] = halves

            def load_rhs3(h):
                r = rhsp.tile([128, 8, 512], BF, name=f"rhs3_{h}", tag="rhs")
                nc.scalar.dma_start(
                    out=r[:],
                    in_=ag_out[(3, h)].rearrange("(t p) n -> p t n", p=128),
                )
                wo_state[(3, h)] = r

            def wo_quarter_gen(qc, wop, quarter):
                """wo GEMM for chunk qc (0..2), one quarter: 16 yields x 2 MMs.
                quarter 0: fps01 += wave0      quarter 1: fps01 += wave1, fout
                quarter 2: fps23 += wave0      quarter 3: fps23 += wave1, fout"""
                q0 = qc * 512
                hp = quarter % 2
                qsp = quarter // 2
                halves = wo_state[(qc, hp)]
                if hp == 0:
                    fps = [
                        wop.tile(
                            [128, 512], F32, name=f"f{qc}_{qsp}_{j}", tag=f"f{j}"
                        )
                        for j in range(2)
                    ]
                    wo_state[(qc, "fps", qsp)] = fps
                fps = wo_state[(qc, "fps", qsp)]
                for i in range(16):
                    ci, jj = i // 2, i % 2
                    hk = 4 * ci + 2 * hp + jj
                    rhs = halves[0] if ci < 4 else halves[1]
                    li = (ci % 4) * 2 + jj
                    for j in range(2):
                        qs = qsp * 2 + j
                        nc.tensor.matmul(
                            fps[j][:],
                            rhs[:, li, qs * 128:(qs + 1) * 128],
                            wo_sb[:, hk, :],
                            start=(hp == 0 and i == 0),
                            stop=(hp == 1 and i == 15),
                        )
                    if i < 15:
                        yield
                if hp == 1:
                    for j in range(2):
                        qs = qsp * 2 + j
                        f_sb = foutp.tile(
                            [128, 512], F32, name=f"fs{qc}_{qsp}_{j}", tag="fs"
                        )
                        nc.scalar.copy(f_sb[:], fps[j][:])
                        nc.sync.dma_start(
                            out=out_ext[q0 + qs * 128:q0 + (qs + 1) * 128, :],
                            in_=f_sb[:],
                        )
                yield

            def wo3_pass(wop, qsp):
                """Last chunk: one fps pair accumulated across 4 head-waves."""
                q0 = 3 * 512
                fps = [
                    wop.tile([128, 512], F32, name=f"f3_{qsp}_{j}", tag=f"f{j}")
                    for j in range(2)
                ]
                for h in range(N_QH):
                    rhs = wo_state[(3, h)]
                    for ci in range(8):
                        hk = 4 * ci + h
                        for j in range(2):
                            qs = qsp * 2 + j
                            nc.tensor.matmul(
                                fps[j][:],
                                rhs[:, ci, qs * 128:(qs + 1) * 128],
                                wo_sb[:, hk, :],
                                start=(h == 0 and ci == 0),
                                stop=(h == N_QH - 1 and ci == 7),
                            )
                for j in range(2):
                    qs = qsp * 2 + j
                    f_sb = foutp.tile(
                        [128, 512], F32, name=f"fs3_{qsp}_{j}", tag="fs"
                    )
                    nc.scalar.copy(f_sb[:], fps[j][:])
                    nc.sync.dma_start(
                        out=out_ext[q0 + qs * 128:q0 + (qs + 1) * 128, :],
                        in_=f_sb[:],
                    )

            def run_filler(f, n=1):
                if f is None:
                    return
                for _ in range(n):
                    try:
                        next(f)
                    except StopIteration:
                        break

            def ag_launch(qc, part):
                nc.gpsimd.collective_compute(
                    "AllGather",
                    mybir.AluOpType.bypass,
                    replica_groups=[list(range(N_CORES))],
                    ins=[ag_in[(qc, part)][:].opt()],
                    outs=[ag_out[(qc, part)][:].opt()],
                )

            class NormTail:
                """Deferred per-head softmax normalization, emitted inside the
                NEXT head's kt stream (kt2: z-reduce + 1/z; kt8: broadcast,
                normalize, store, collective launch + gathered-wave DMA)."""

                def __init__(self, qc, h, pv, z_acc):
                    self.qc, self.h, self.pv, self.z_acc = qc, h, pv, z_acc

                def stage_a(self):
                    qc, h = self.qc, self.h
                    zr = cur_pools["pzb"].tile([1, 512], F32, name=f"zr{qc}_{h}", tag="zb")
                    nc.tensor.matmul(
                        zr[:], onesc_sb[:], self.z_acc[:], start=True, stop=True
                    )
                    lnz = smallp.tile([1, 512], F32, name=f"ln{qc}_{h}", tag="lnz")
                    nc.scalar.activation(
                        out=lnz[:], in_=zr[:],
                        func=mybir.ActivationFunctionType.Ln,
                    )
                    self.invz = smallp.tile([1, 512], BF, name=f"iz{qc}_{h}", tag="iz")
                    nc.scalar.activation(
                        out=self.invz[:], in_=lnz[:],
                        func=mybir.ActivationFunctionType.Exp, scale=-1.0,
                    )

                def stage_b(self):
                    qc, h = self.qc, self.h
                    bc = cur_pools["pzb"].tile([128, 512], F32, name=f"bc{qc}_{h}", tag="zb")
                    nc.tensor.matmul(
                        bc[:], onesr_sb[:], self.invz[:], start=True, stop=True
                    )
                    bc_sb = smallp.tile([128, 512], BF, name=f"bcs{qc}_{h}", tag="bcs")
                    nc.scalar.copy(bc_sb[:], bc[:])
                    at_sb = atp.tile([128, 512], BF, name=f"at{qc}_{h}", tag="at")
                    nc.vector.tensor_mul(at_sb[:], self.pv[:], bc_sb[:])
                    if qc < 3:
                        hp, j = h // 2, h % 2
                        nc.gpsimd.dma_start(
                            out=ag_in[(qc, hp)][j * 128:(j + 1) * 128, :],
                            in_=at_sb[:],
                        )
                        if j == 1:
                            ag_launch(qc, hp)
                            if hp == 1:
                                # wave1 halves: ring slot is free by now
                                load_rhs_halves(qc, 1)
                    else:
                        nc.gpsimd.dma_start(out=ag_in[(3, h)][:], in_=at_sb[:])
                        ag_launch(3, h)
                        load_rhs3(h)

            def emit_ev(qc, h, kt, stps, pv, z_acc):
                e_t = ep.tile([128, 512], BF, name=f"e{qc}_{h}_{kt}", tag="e")
                nc.scalar.activation(
                    out=e_t[:], in_=stps[kt][:],
                    func=mybir.ActivationFunctionType.Exp,
                )
                nc.tensor.matmul(
                    pv[:], v_sb[:, kt, :], e_t[:],
                    start=(kt == 0), stop=(kt == N_KT - 1),
                )
                if kt == 0:
                    nc.vector.tensor_copy(z_acc[:], e_t[:])
                else:
                    nc.vector.tensor_add(z_acc[:], z_acc[:], e_t[:])

            def attn_head(qc, h, filler=None, prev_tail=None):
                q0 = qc * 512
                pv = cur_pools["ppv"].tile([128, 512], F32, name=f"pv{qc}_{h}", tag=f"pv{h % 2}")
                z_acc = zpool.tile([128, 512], F32, name=f"z{qc}_{h}", tag="zacc")
                stps = {}
                for kt in range(N_KT):
                    if prev_tail is not None:
                        if kt == 2:
                            prev_tail.stage_a()
                        elif kt == 8:
                            prev_tail.stage_b()
                    stp = cur_pools["psc"].tile([128, 512], F32, name=f"st{qc}_{h}_{kt}", tag="st")
                    stps[kt] = stp
                    nc.tensor.matmul(
                        stp[:],
                        kt_sb[:, kt * 128:(kt + 1) * 128],
                        qt_sb[:, h, q0:q0 + 512],
                        start=True, stop=True,
                    )
                    run_filler(filler)
                    if kt > 0:
                        emit_ev(qc, h, kt - 1, stps, pv, z_acc)
                emit_ev(qc, h, N_KT - 1, stps, pv, z_acc)
                run_filler(filler)
                return NormTail(qc, h, pv, z_acc)

            # ================= phase 1: projections chunks 0-2 =============
            with tc.tile_pool(name="wqp", bufs=1) as wqp:
                wq_sb = wqp.tile([128, N_KC, 512], BF)
                for ch in range(4):
                    nc.sync.dma_start(
                        out=wq_sb[:, ch * 8:(ch + 1) * 8, :],
                        in_=wqt[:, ch * 8:(ch + 1) * 8, :],
                    )
                with tc.tile_pool(name="pprojA", bufs=4, space="PSUM") as pprojA:
                    pprojA._ropetag = "pacc"
                    for c in range(3):
                        scope = nc.named_scope(f"proj{c}"); scope.__enter__()
                        load_xt_group(c + 1, 0)
                        load_xt_group(c + 1, 1)
                        proj_pair(pprojA, pprojA, c, [("k", 0), ("v", 0)])
                        load_xt_group(c + 1, 2)
                        load_xt_group(c + 1, 3)
                        proj_pair(pprojA, pprojA, c, [("q", 0), ("q", 1)])
                        for g in range(4, 8):
                            load_xt_group(c + 1, g)
                        proj_pair(pprojA, pprojA, c, [("q", 2), ("q", 3)])
                        scope.__exit__(None, None, None)

                # ============ phase 2: c3 K/V + attention chunk 0 ==========
                # (pprojA closed; fresh 8-bank layout: psc 3 + ppv 2 + pzb 1
                #  + pprojB 2)
                with (
                    tc.tile_pool(name="pscA", bufs=3, space="PSUM") as psc,
                    tc.tile_pool(name="ppvA", bufs=1, space="PSUM") as ppv,
                    tc.tile_pool(name="pzbA", bufs=1, space="PSUM") as pzb,
                    tc.tile_pool(name="pprojB", bufs=2, space="PSUM") as pprojB,
                ):
                    pzb._ropetag = "zb"
                    cur_pools["psc"], cur_pools["ppv"], cur_pools["pzb"] = (
                        psc, ppv, pzb
                    )
                    scope = nc.named_scope("proj3"); scope.__enter__()
                    proj_pair(pprojB, pzb, 3, [("k", 0), ("v", 0)])
                    scope.__exit__(None, None, None)

                    scope = nc.named_scope("attn0"); scope.__enter__()
                    gA = proj_pair_gen(pprojB, pzb, 3, [("q", 0), ("q", 1)])
                    gB = proj_pair_gen(pprojB, pzb, 3, [("q", 2), ("q", 3)])
                    tail = None
                    for h, g in ((0, gA), (1, gA), (2, gB), (3, gB)):
                        tail = attn_head(0, h, filler=g, prev_tail=tail)
                    # flush the last head's tail inside this pool scope
                    tail.stage_a()
                    tail.stage_b()
                    tail = None
                    flush_rope(pzb)
                    load_rhs_halves(0, 0)
                    scope.__exit__(None, None, None)

            # ======== phase 3: attention chunks 1-3 + wo ===========
            with (
                tc.tile_pool(name="wop", bufs=1) as wop_s,
                tc.tile_pool(name="pscB", bufs=3, space="PSUM") as psc,
                tc.tile_pool(name="ppvB", bufs=1, space="PSUM") as ppv,
                tc.tile_pool(name="pzbB", bufs=1, space="PSUM") as pzb,
                tc.tile_pool(name="pwo", bufs=1, space="PSUM") as pwo,
            ):
                cur_pools["psc"], cur_pools["ppv"], cur_pools["pzb"] = (
                    psc, ppv, pzb
                )
                wo_sb = wop_s.tile([128, N_KC, 512], BF)
                for ch in range(4):
                    nc.gpsimd.dma_start(
                        out=wo_sb[:, ch * 8:(ch + 1) * 8, :],
                        in_=wot[:, ch * 8:(ch + 1) * 8, :],
                    )
                pending = [None]
                for qc in range(1, N_TC):
                    scope = nc.named_scope(f"attn{qc}"); scope.__enter__()
                    if qc >= 2:
                        pending.append(wo_quarter_gen(qc - 2, pwo, 3))
                    for q in range(3):
                        pending.append(wo_quarter_gen(qc - 1, pwo, q))
                    for h in range(N_QH):
                        f = pending.pop(0) if pending else None
                        tail = attn_head(qc, h, filler=f, prev_tail=tail)
                        run_filler(f, 16)
                    # wave0 halves of this chunk's gather (launched
                    # mid-block) load during the next block's head 0
                    if qc < 3:
                        load_rhs_halves(qc, 0)
                    scope.__exit__(None, None, None)
                scope = nc.named_scope("wo3"); scope.__enter__()
                tail.stage_a()
                tail.stage_b()
                pending.append(wo_quarter_gen(N_TC - 2, pwo, 3))
                for f in pending:   # quarter 3 of chunk 2
                    run_filler(f, 17)
                wo3_pass(pwo, 0)
                wo3_pass(pwo, 1)
                scope.__exit__(None, None, None)

    _split_multi_waits(nc)
    return nc


def _host_prep(x, cos, sin, wq, wk, wv, wo):
    scale = np.float32(HD ** -0.5)
    perm = np.concatenate([np.arange(0, HD, 2), np.arange(1, HD, 2)])

    xt = np.ascontiguousarray(x.T.reshape(N_KC, 128, S)).astype(_BF16)
    cosT = cos.T.astype(np.float32)
    sinT = sin.T.astype(np.float32)
    cs1 = np.concatenate([cosT, sinT], axis=0).astype(_BF16)
    cs2 = np.concatenate([sinT, cosT], axis=0).astype(_BF16)

    m1 = np.zeros((HD, HD), np.float32)
    m1[np.arange(64), np.arange(64)] = 1.0
    m1[np.arange(64) + 64, np.arange(64)] = -1.0
    m2 = np.zeros((HD, HD), np.float32)
    m2[np.arange(64), np.arange(64) + 64] = 1.0
    m2[np.arange(64) + 64, np.arange(64) + 64] = 1.0

    def to_tiles(wT, ncols):
        return np.ascontiguousarray(
            wT.reshape(N_KC, 128, ncols).transpose(1, 0, 2)
        ).astype(_BF16)

    shared = {
        "xt": xt,
        "cs1": cs1,
        "cs2": cs2,
        "mix1": m1.astype(_BF16),
        "mix2": m2.astype(_BF16),
        "onesc": np.ones((HD, 1), np.float32),
        "onesr": np.ones((1, HD), np.float32).astype(_BF16),
    }
    in_maps = []
    for c in range(N_CORES):
        wq_c = wq[c * 512:(c + 1) * 512].reshape(N_QH, HD, D)[:, perm, :]
        wq_c = wq_c.reshape(512, D) * scale
        wk_c = wk[c * HD:(c + 1) * HD][perm, :]
        wv_c = wv[c * HD:(c + 1) * HD]
        wo_c = wo[c * 512:(c + 1) * 512]
        m = dict(shared)
        m["wqt"] = to_tiles(np.ascontiguousarray(wq_c.T), 512)
        m["wkt"] = to_tiles(np.ascontiguousarray(wk_c.T), HD)
        m["wvt"] = to_tiles(np.ascontiguousarray(wv_c.T), HD)
        m["wot"] = to_tiles(np.ascontiguousarray(wo_c.T), 512)
        in_maps.append(m)
    return in_maps


def kernel(x, cos, sin, wq, wk, wv, wo, _trace=False):
    x = np.asarray(x, np.float32)
    cos = np.asarray(cos, np.float32)
    sin = np.asarray(sin, np.float32)
    wq = np.asarray(wq, np.float32)
    wk = np.asarray(wk, np.float32)
    wv = np.asarray(wv, np.float32)
    wo = np.asarray(wo, np.float32)

    in_maps = _host_prep(x, cos, sin, wq, wk, wv, wo)
    if "nc" not in _NC_CACHE:
        _NC_CACHE["nc"] = _build()
    nc = _NC_CACHE["nc"]
    res = run_bass_kernel_spmd(
        nc, in_maps, core_ids=list(range(N_CORES)), trace=_trace
    )
    out = np.concatenate([res.results[c]["out"] for c in range(N_CORES)], axis=1)
    out = np.ascontiguousarray(out, dtype=np.float32)
    if _trace:
        kernel._last_exec_time_ns = res.exec_time_ns
        kernel._last_result = res
    return out
